# revision 14
# baseline (speedup 1.0000x reference)
"""Causal-intervention attention on 8 trn2 cores.

Sharding: head-parallel. Core c computes heads {2c, 2c+1} for BOTH batches.
Each core emits a partial output y_c = ctx_c @ Wo[rows_c] in fp16; the host
sums the 8 partials and adds the (folded) bias.

Mask handling: tokens are sorted by cause_mask on the host (per batch).
scores * (1 - 0.5*s*cm[q]*em[k]) is exact by using an em-scaled copy of K^T
(K2) for cm=1 queries and plain K^T for cm=0 queries. Query slices are a
uniform 512 wide; the cm boundary splits only the scores matmul col-range.

Per-unit structure (unit = (q-slice j, key-tile kt)): both heads' scores go
into ONE [128, 1024] PSUM tile (h0 cols 0:512, h1 cols 512:1024) via
tile_position (0,0)/(64,0), so a single full-width exp serves both heads.
Score PSUM is double-buffered so PE never waits on ACT.

Softmax: exp without max subtraction (scores ~ N(0,1), safe in fp32).
Denominator from a ones column appended to V (M=65 AV matmul). The
reciprocal row is broadcast across partitions with a K=1 PE outer-product
(no DRAM bounce). Normalization is applied to the attention output before
the output projection; h1's rows are DMA-shifted to partitions 64:128.

All matmul operands bf16; accumulation fp32 in PSUM.
"""

import numpy as np
import ml_dtypes
from contextlib import ExitStack

import concourse.bass as bass
from concourse import bacc
import concourse.mybir as mybir
import concourse.tile as tile
from concourse.bass_utils import run_bass_kernel_spmd

B, S, D, H = 2, 2048, 1024, 16
HD = D // H  # 64
NCORES = 8
HPC = H // NCORES  # heads per core = 2
DPC = HPC * HD  # head-dim cols per core = 128
P = 128

F32 = mybir.dt.float32
F16 = mybir.dt.float16
BF16 = mybir.dt.bfloat16

NKT = S // P            # 16 key tiles of 128
NSL = S // 512          # 4 query slices of 512
VSTRIDE = 2 * (HD + 1)  # V' storage per k-tile: [V_h0|1|V_h1|1] = 130

LAST_RESULTS = None  # BassKernelResults of the most recent run (for test.py)
TRACE = False


def _ranges(j, boundary):
    """Column ranges of q-slice j=[512j, 512j+512) split at the cm=1/cm=0
    boundary. Returns [(c0, c1, use_k2)]."""
    q0, q1 = 512 * j, 512 * (j + 1)
    b = min(max(boundary, q0), q1)
    out = []
    if b > q0:
        out.append((q0, b, True))
    if q1 > b:
        out.append((b, q1, False))
    return out


def _build_program(n_c1, repeat=1):
    """n_c1: tuple of per-batch cm=1 counts (compile-time structure).
    repeat>1 re-runs the whole computation (timing experiments only)."""
    nc = bacc.Bacc(None, target_bir_lowering=False)

    xtb = nc.declare_dram_parameter("xtb", [B, D, S], BF16, isOutput=False)
    wq = nc.declare_dram_parameter("wq", [D, DPC], BF16, isOutput=False)
    wk = nc.declare_dram_parameter("wk", [D, DPC], BF16, isOutput=False)
    wvb = nc.declare_dram_parameter("wvb", [D, DPC], BF16, isOutput=False)
    wo = nc.declare_dram_parameter("wo", [DPC, D], BF16, isOutput=False)
    bq = nc.declare_dram_parameter("bq", [DPC, 1], F32, isOutput=False)
    bk = nc.declare_dram_parameter("bk", [DPC, 1], F32, isOutput=False)
    emf = nc.declare_dram_parameter("emf", [B, P, S], BF16, isOutput=False)
    y = nc.declare_dram_parameter("y", [B, S, D], F16, isOutput=True)

    with tile.TileContext(nc) as tc, ExitStack() as ctx:
        # ---- pools ----
        xtb_pool = ctx.enter_context(tc.tile_pool(name="xtb", bufs=16))
        wpool = ctx.enter_context(tc.tile_pool(name="w", bufs=1))
        emf_pool = ctx.enter_context(tc.tile_pool(name="emf", bufs=2))
        qk_pool = ctx.enter_context(tc.tile_pool(name="qk", bufs=2))
        v_pool = ctx.enter_context(tc.tile_pool(name="v", bufs=2))
        et_pool = ctx.enter_context(tc.tile_pool(name="et", bufs=4))
        small = ctx.enter_context(tc.tile_pool(name="small", bufs=3))
        y_pool = ctx.enter_context(tc.tile_pool(name="y", bufs=3))
        ps_s = ctx.enter_context(tc.tile_pool(name="pss", bufs=2, space="PSUM"))
        ps_o = ctx.enter_context(tc.tile_pool(name="pso", bufs=1, space="PSUM"))
        ps_proj = ctx.enter_context(tc.tile_pool(name="psp", bufs=2, space="PSUM"))

        # ---- weights / biases / constants (once) ----
        wq_sb = wpool.tile([P, D], BF16, tag="wq_sb")
        wk_sb = wpool.tile([P, D], BF16, tag="wk_sb")
        wv_sb = wpool.tile([P, D], BF16, tag="wv_sb")
        wo_sb = wpool.tile([P, D], BF16, tag="wo_sb")
        bq_sb = wpool.tile([P, 1], F32, tag="bq_sb")
        bk_sb = wpool.tile([P, 1], F32, tag="bk_sb")
        ones_sb = wpool.tile([P, HD], BF16, tag="ones_sb")
        nc.gpsimd.dma_start(
            out=wq_sb[:].rearrange("p (t m) -> p t m", t=D // P),
            in_=wq.rearrange("(t p) m -> p t m", p=P))
        nc.gpsimd.dma_start(
            out=wk_sb[:].rearrange("p (t m) -> p t m", t=D // P),
            in_=wk.rearrange("(t p) m -> p t m", p=P))
        nc.gpsimd.dma_start(
            out=wv_sb[:].rearrange("p (t m) -> p t m", t=D // P),
            in_=wvb.rearrange("(t p) m -> p t m", p=P))
        nc.gpsimd.dma_start(out=wo_sb[:], in_=wo[:])
        nc.gpsimd.dma_start(out=bq_sb[:], in_=bq[:])
        nc.gpsimd.dma_start(out=bk_sb[:], in_=bk[:])
        nc.vector.memset(ones_sb[:], 1.0)
        wqs = [wq_sb[:, kt * DPC:(kt + 1) * DPC] for kt in range(D // P)]
        wks = [wk_sb[:, kt * DPC:(kt + 1) * DPC] for kt in range(D // P)]
        wvbs = [wv_sb[:, kt * DPC:(kt + 1) * DPC] for kt in range(D // P)]

        for rep in range(repeat):

            def gen_load(b):
                """DMA x^T + emf for batch b; returns tiles."""
                xtbs = []
                for kt in range(D // P):
                    tb = xtb_pool.tile([P, S], BF16, tag="xtb",
                                       name=f"xtb_{rep}_{b}_{kt}")
                    nc.sync.dma_start(out=tb[:],
                                      in_=xtb[b, kt * P:(kt + 1) * P, :])
                    xtbs.append(tb)
                emf_sb = emf_pool.tile([P, S], BF16, tag="emf_sb",
                                       name=f"emf_{rep}_{b}")
                nc.sync.dma_start(out=emf_sb[:], in_=emf[b])
                return xtbs, emf_sb

            def gen_proj(b, xtbs, emf_sb, bt):
                """Q/K/V projections for batch b as labeled pieces.

                Yields after each piece: ('K', ns), ('V', st), ('Q', ns).
                Emission order: K0, Q0, V0, V1, K1, V2..5, K2, V6..9, K3,
                V10..15, Q1..Q3 — so attention can start early.
                """
                def k_piece(ns):
                    cs = slice(ns * 512, (ns + 1) * 512)
                    psk = ps_proj.tile([P, 512], F32, tag="pp",
                                       name=f"psk_{rep}_{b}_{ns}")
                    for kt in range(D // P):
                        nc.tensor.matmul(
                            psk[:], lhsT=wks[kt], rhs=xtbs[kt][:, cs],
                            start=(kt == 0), stop=(kt == D // P - 1))
                    nc.vector.tensor_scalar_add(bt["kt"][:, cs], psk[:], bk_sb[:])
                    nc.vector.scalar_tensor_tensor(
                        out=bt["k2t"][:, cs], in0=psk[:], scalar=bk_sb[:],
                        in1=emf_sb[:, cs],
                        op0=mybir.AluOpType.add, op1=mybir.AluOpType.mult)

                def q_piece(ns):
                    cs = slice(ns * 512, (ns + 1) * 512)
                    psq = ps_proj.tile([P, 512], F32, tag="pp",
                                       name=f"psq_{rep}_{b}_{ns}")
                    for kt in range(D // P):
                        nc.tensor.matmul(
                            psq[:], lhsT=wqs[kt], rhs=xtbs[kt][:, cs],
                            start=(kt == 0), stop=(kt == D // P - 1))
                    nc.vector.tensor_scalar_add(bt["qt"][:, cs], psq[:], bq_sb[:])

                def v_piece(st):
                    psv = ps_proj.tile([P, 512], F32, tag="pp",
                                       name=f"psv_{rep}_{b}_{st}")
                    for kt in range(D // P):
                        nc.tensor.matmul(
                            psv[:, 0:DPC],
                            lhsT=xtbs[kt][:, st * P:(st + 1) * P],
                            rhs=wvbs[kt],
                            start=(kt == 0), stop=(kt == D // P - 1))
                    o = st * VSTRIDE
                    nc.vector.tensor_copy(bt["v"][:, o:o + HD], psv[:, 0:HD])
                    nc.vector.tensor_copy(
                        bt["v"][:, o + HD + 1:o + 2 * HD + 1],
                        psv[:, HD:2 * HD])

                nc.vector.memset(bt["v"][:], 1.0)
                k_piece(0); yield ("K", 0)
                q_piece(0); yield ("Q", 0)
                v_piece(0); yield ("V", 0)
                v_piece(1); yield ("V", 1)
                nv = 2
                for ns in (1, 2, 3):
                    k_piece(ns); yield ("K", ns)
                    for _ in range(4):
                        if nv < NKT:
                            v_piece(nv); yield ("V", nv)
                            nv += 1
                while nv < NKT:
                    v_piece(nv); yield ("V", nv)
                    nv += 1
                for ns in (1, 2, 3):
                    q_piece(ns); yield ("Q", ns)

            def gen_attn(b, bt):
                """Attention units + lagged output projection, as a generator.

                Yields (j, kt) before emitting unit (j, kt)'s scores so the
                driver can pump projection pieces first.
                """
                qt_sb, kt_sb, k2t_sb = bt["qt"], bt["kt"], bt["k2t"]
                ont_sb, v_sb = bt["ont"], bt["v"]
                units = [(j, kt) for j in range(NSL) for kt in range(NKT)]
                pend = {}
                o_ps = {}
                state = {"st_ready": 0, "st_emitted": 0}
                ready_log = []

                def emit_scores(u):
                    j, kt = u
                    q0 = 512 * j
                    ks = slice(kt * P, (kt + 1) * P)
                    sp = ps_s.tile([P, 1024], F32, tag="sp",
                                   name=f"sp_{rep}_{b}_{j}_{kt}")
                    for (c0, c1, use_k2) in _ranges(j, n_c1[b]):
                        src = k2t_sb if use_k2 else kt_sb
                        nc.tensor.matmul(
                            sp[:, c0 - q0:c1 - q0],
                            lhsT=src[0:HD, ks], rhs=qt_sb[0:HD, c0:c1],
                            start=True, stop=True, tile_position=(0, 0))
                        nc.tensor.matmul(
                            sp[:, 512 + c0 - q0:512 + c1 - q0],
                            lhsT=src[HD:P, ks], rhs=qt_sb[HD:P, c0:c1],
                            start=True, stop=True, tile_position=(64, 0))
                    e = et_pool.tile([P, 1024], BF16, tag="e",
                                     name=f"e_{rep}_{b}_{j}_{kt}")
                    nc.scalar.activation(e[:], sp[:],
                                         mybir.ActivationFunctionType.Exp)
                    pend[u] = e

                def emit_av(u):
                    j, kt = u
                    e = pend.pop(u)
                    if j not in o_ps:
                        o0_t = ps_o.tile([HD + 1, 512], F32, tag="o0",
                                         name=f"o0_{rep}_{b}_{j}")
                        o1_t = ps_o.tile([HD + 1, 512], F32, tag="o1",
                                         name=f"o1_{rep}_{b}_{j}")
                        o_ps[j] = (o0_t, o1_t)
                    o0, o1 = o_ps[j]
                    vo = kt * VSTRIDE
                    nc.tensor.matmul(
                        o0[0:HD + 1, :],
                        lhsT=v_sb[:, vo:vo + HD + 1], rhs=e[:, 0:512],
                        start=(kt == 0), stop=(kt == NKT - 1))
                    nc.tensor.matmul(
                        o1[0:HD + 1, :],
                        lhsT=v_sb[:, vo + HD + 1:vo + 2 * (HD + 1)],
                        rhs=e[:, 512:1024],
                        start=(kt == 0), stop=(kt == NKT - 1))
                    if kt == NKT - 1:
                        emit_norm(j)

                def emit_norm(j):
                    # normalize q-slice j: rows 0:64 of o_h /= row 64 (denom).
                    # recip row -> PE K=1 outer product broadcasts it across
                    # 64 partitions (no DRAM bounce). h0's mul lands directly
                    # in ont rows 0:64; h1's is DMA-shifted to rows 64:128.
                    q0 = 512 * j
                    o0, o1 = o_ps.pop(j)
                    for h, op in ((0, o0), (1, o1)):
                        r = small.tile([HD + 1, 512], BF16, tag=f"r{h}",
                                       name=f"r{h}_{rep}_{b}_{j}")
                        with nc.allow_low_precision(
                                reason="bf16 softmax denom recip: ~0.2%, "
                                       "within the 2e-2 budget"):
                            nc.vector.reciprocal(r[HD:HD + 1, :],
                                                 op[HD:HD + 1, :])
                        bc = ps_proj.tile([P, 512], F32, tag="pp",
                                          name=f"bc{h}_{rep}_{b}_{j}")
                        nc.tensor.matmul(
                            bc[0:HD, :], lhsT=ones_sb[HD:HD + 1, 0:HD],
                            rhs=r[HD:HD + 1, :], start=True, stop=True,
                            tile_position=(64, 0))
                        bcs = small.tile([HD, 512], BF16, tag=f"bcs{h}",
                                         name=f"bcs{h}_{rep}_{b}_{j}")
                        nc.vector.tensor_copy(bcs[:], bc[0:HD, :])
                        if h == 0:
                            nc.vector.tensor_mul(
                                ont_sb[0:HD, q0:q0 + 512],
                                op[0:HD, :], bcs[:])
                        else:
                            tmp = small.tile([HD, 512], BF16, tag="tmp1",
                                             name=f"tmp1_{rep}_{b}_{j}")
                            nc.vector.tensor_mul(
                                tmp[:], op[0:HD, :], bcs[:])
                            nc.gpsimd.dma_start(
                                out=ont_sb[HD:P, q0:q0 + 512],
                                in_=tmp[:])
                    state["st_ready"] = (q0 + 512) // P

                def emit_oproj(upto):
                    while state["st_emitted"] < upto:
                        st = state["st_emitted"]
                        ysb = y_pool.tile([P, D], F16, tag="ysb",
                                          name=f"ysb_{rep}_{b}_{st}")
                        for half in range(2):
                            yp = ps_proj.tile([P, 512], F32, tag="pp",
                                              name=f"yp_{rep}_{b}_{st}_{half}")
                            nc.tensor.matmul(
                                yp[:],
                                lhsT=ont_sb[:, st * P:(st + 1) * P],
                                rhs=wo_sb[:, half * 512:(half + 1) * 512],
                                start=True, stop=True)
                            nc.vector.tensor_copy(
                                ysb[:, half * 512:(half + 1) * 512], yp[:])
                        nc.gpsimd.dma_start(out=y[b, st * P:(st + 1) * P, :],
                                          in_=ysb[:])
                        state["st_emitted"] += 1

                for i in range(len(units) + 1):
                    if i < len(units):
                        yield units[i]
                        emit_scores(units[i])
                    if i > 0:
                        emit_av(units[i - 1])
                        ready_log.append((i, state["st_ready"]))
                        lagged = max((s for ii, s in ready_log if ii <= i - 2),
                                     default=0)
                        emit_oproj(lagged)
                emit_oproj(state["st_ready"])

            def batch_tiles(b):
                return {
                    "qt": qk_pool.tile([P, S], BF16, tag="qt_sb",
                                       name=f"qt_{rep}_{b}"),
                    "kt": qk_pool.tile([P, S], BF16, tag="kt_sb",
                                       name=f"kt_{rep}_{b}"),
                    "k2t": qk_pool.tile([P, S], BF16, tag="k2t_sb",
                                        name=f"k2t_{rep}_{b}"),
                    "ont": qk_pool.tile([P, S], BF16, tag="ont_sb",
                                        name=f"ont_{rep}_{b}"),
                    "v": v_pool.tile([P, NKT * VSTRIDE], BF16, tag="v_sb",
                                     name=f"v_{rep}_{b}"),
                }

            # ---- batch 0: load, then proj pieces pumped just-in-time so
            # attention starts as soon as K0/Q0/V0 are projected ----
            xtbs0, emf0 = gen_load(0)
            bt0 = batch_tiles(0)
            if rep == 0:
                # PE p-state warm-up: ~8.5us of dummy matmuls inside the
                # x-load window so the first projection runs at full clock.
                warm = ps_proj.tile([P, 512], F32, tag="pp", name="warm")
                for wi in range(24):
                    nc.tensor.matmul(warm[:], lhsT=wq_sb[:, 0:P],
                                     rhs=wq_sb[:, 0:512],
                                     start=(wi == 0), stop=(wi == 23))
                warm_rd = small.tile([1, 1], F32, tag="warm_rd")
                nc.vector.tensor_copy(warm_rd[:], warm[0:1, 0:1])
            proj0 = gen_proj(0, xtbs0, emf0, bt0)
            done0 = {"K": set(), "V": set(), "Q": set()}

            def pump(gen, done):
                p = next(gen, None)
                if p is None:
                    return False
                done[p[0]].add(p[1])
                return True

            def prereq_met(done, j, kt):
                return (kt // 4 in done["K"] and j in done["Q"]
                        and min(kt + 1, NKT - 1) in done["V"])

            # batch 1 resources, loaded/projected during batch 0's attention
            bt1 = batch_tiles(1)
            state1 = {"loaded": False, "proj": None, "done": False,
                      "d1": {"K": set(), "V": set(), "Q": set()}}

            attn0 = gen_attn(0, bt0)
            for ui, u in enumerate(attn0):
                j, kt = u
                while not prereq_met(done0, j, kt):
                    if not pump(proj0, done0):
                        break
                if ui >= 4 and not state1["loaded"]:
                    xtbs1, emf1 = gen_load(1)
                    state1["loaded"] = True
                    state1["proj"] = gen_proj(1, xtbs1, emf1, bt1)
                b0_mostly_done = (len(done0["K"]) == 4
                                  and len(done0["V"]) == NKT
                                  and 1 in done0["Q"])
                if ui >= 12 and ui % 2 == 0 and state1["loaded"] \
                        and not state1["done"] and b0_mostly_done:
                    state1["done"] = not pump(state1["proj"], state1["d1"])
            while pump(proj0, done0):
                pass
            if not state1["loaded"]:
                xtbs1, emf1 = gen_load(1)
                state1["proj"] = gen_proj(1, xtbs1, emf1, bt1)
            while pump(state1["proj"], state1["d1"]):
                pass

            # ---- batch 1 attention ----
            attn1 = gen_attn(1, bt1)
            for _ in attn1:
                pass

    return nc


def _host_prep(x, cause_mask, effect_mask, intervention_strength,
               Wq, bq, Wk, bk, Wv, bv, Wo, bo):
    x = np.asarray(x, dtype=np.float32)
    cause_mask = np.asarray(cause_mask).astype(bool)
    effect_mask = np.asarray(effect_mask).astype(bool)
    s_int = float(np.asarray(intervention_strength))
    Wq = np.asarray(Wq, np.float32); bq = np.asarray(bq, np.float32)
    Wk = np.asarray(Wk, np.float32); bk = np.asarray(bk, np.float32)
    Wv = np.asarray(Wv, np.float32); bv = np.asarray(bv, np.float32)
    Wo = np.asarray(Wo, np.float32); bo = np.asarray(bo, np.float32)

    # host prep: sort tokens by cause_mask (descending) per batch
    perms, n_c1 = [], []
    for b in range(B):
        p = np.argsort(~cause_mask[b], kind="stable")
        perms.append(p)
        n_c1.append(int(cause_mask[b].sum()))
    xp = np.stack([x[b][perms[b]] for b in range(B)])          # [B, S, D]
    xt = np.ascontiguousarray(xp.transpose(0, 2, 1))           # [B, D, S]
    xtb = xt.astype(ml_dtypes.bfloat16)
    emfac = np.stack([
        1.0 - 0.5 * s_int * effect_mask[b][perms[b]].astype(np.float32)
        for b in range(B)])                                    # [B, S]
    emf = np.ascontiguousarray(
        np.broadcast_to(emfac[:, None, :], (B, P, S))).astype(ml_dtypes.bfloat16)

    scale = 1.0 / np.sqrt(np.float32(HD))

    in_maps = []
    for c in range(NCORES):
        cols = slice(c * DPC, (c + 1) * DPC)
        in_maps.append({
            "xtb": xtb, "emf": emf,
            "wq": np.ascontiguousarray(Wq[:, cols] * scale).astype(ml_dtypes.bfloat16),
            "wk": np.ascontiguousarray(Wk[:, cols]).astype(ml_dtypes.bfloat16),
            "wvb": np.ascontiguousarray(Wv[:, cols]).astype(ml_dtypes.bfloat16),
            "wo": np.ascontiguousarray(Wo[cols, :]).astype(ml_dtypes.bfloat16),
            "bq": np.ascontiguousarray((bq[cols] * scale).reshape(DPC, 1)),
            "bk": np.ascontiguousarray(bk[cols].reshape(DPC, 1)),
        })

    bo_eff = bo + bv @ Wo
    return in_maps, perms, tuple(n_c1), bo_eff


def kernel(x, cause_mask, effect_mask, intervention_strength,
           Wq, bq, Wk, bk, Wv, bv, Wo, bo):
    global LAST_RESULTS
    in_maps, perms, n_c1, bo_eff = _host_prep(
        x, cause_mask, effect_mask, intervention_strength,
        Wq, bq, Wk, bk, Wv, bv, Wo, bo)
    nc = _build_program(n_c1)
    nc.finalize()
    LAST_RESULTS = run_bass_kernel_spmd(
        nc, in_maps, core_ids=list(range(NCORES)), trace=TRACE)

    y = np.zeros((B, S, D), np.float32)
    for c in range(NCORES):
        y += LAST_RESULTS.results[c]["y"].astype(np.float32)
    y += bo_eff[None, None, :]
    out = np.empty_like(y)
    for b in range(B):
        out[b][perms[b]] = y[b]  # undo the token sort
    return out


# revision 28
# speedup vs baseline: 1.0177x; 1.0177x over previous
"""Causal-intervention attention on 8 trn2 cores.

Sharding: head-parallel. Core c computes heads {2c, 2c+1} for BOTH batches.
Each core emits a partial output y_c = ctx_c @ Wo[rows_c] in fp16; the host
sums the 8 partials and adds the (folded) bias.

Mask handling: tokens are sorted by cause_mask on the host (per batch).
scores * (1 - 0.5*s*cm[q]*em[k]) is exact by using an em-scaled copy of K^T
(K2) for cm=1 queries and plain K^T for cm=0 queries. Query slices are a
uniform 512 wide; the cm boundary splits only the scores matmul col-range.

Per-unit structure (unit = (q-slice j, key-tile kt)): both heads' scores go
into ONE [128, 1024] PSUM tile (h0 cols 0:512, h1 cols 512:1024) via
tile_position (0,0)/(64,0), so a single full-width exp serves both heads.
Score PSUM is double-buffered so PE never waits on ACT.

Softmax: exp without max subtraction (scores ~ N(0,1), safe in fp32).
Denominator from a ones column appended to V (M=65 AV matmul). The
reciprocal row is broadcast across partitions with a K=1 PE outer-product
(no DRAM bounce). Normalization is applied to the attention output before
the output projection; h1's rows are DMA-shifted to partitions 64:128.

All matmul operands bf16; accumulation fp32 in PSUM.
"""

import numpy as np
import ml_dtypes
from contextlib import ExitStack

import concourse.bass as bass
from concourse import bacc
import concourse.mybir as mybir
import concourse.tile as tile
from concourse.bass_utils import run_bass_kernel_spmd

B, S, D, H = 2, 2048, 1024, 16
HD = D // H  # 64
NCORES = 8
HPC = H // NCORES  # heads per core = 2
DPC = HPC * HD  # head-dim cols per core = 128
P = 128

F32 = mybir.dt.float32
F16 = mybir.dt.float16
BF16 = mybir.dt.bfloat16

NKT = S // P            # 16 key tiles of 128
NSL = S // 512          # 4 query slices of 512
VSTRIDE = 2 * (HD + 1)  # V' storage per k-tile: [V_h0|1|V_h1|1] = 130

LAST_RESULTS = None  # BassKernelResults of the most recent run (for test.py)
TRACE = False

# schedule pacing knobs (sim-swept)
PUMP_START = 12   # unit index to start pumping batch-1 proj pieces
PUMP_MOD = 1      # pump one piece every PUMP_MOD units
WARM = 24         # PE p-state warm-up matmuls
LOAD1_AT = 4      # unit index to issue batch-1 x/emf loads
LAG = 2           # oproj lag in units


def _ranges(j, boundary):
    """Column ranges of q-slice j=[512j, 512j+512) split at the cm=1/cm=0
    boundary. Returns [(c0, c1, use_k2)]."""
    q0, q1 = 512 * j, 512 * (j + 1)
    b = min(max(boundary, q0), q1)
    out = []
    if b > q0:
        out.append((q0, b, True))
    if q1 > b:
        out.append((b, q1, False))
    return out


def _build_program(n_c1, repeat=1):
    """n_c1: tuple of per-batch cm=1 counts (compile-time structure).
    repeat>1 re-runs the whole computation (timing experiments only)."""
    nc = bacc.Bacc(None, target_bir_lowering=False)

    xtb = nc.declare_dram_parameter("xtb", [B, D, S], BF16, isOutput=False)
    wq = nc.declare_dram_parameter("wq", [D, DPC], BF16, isOutput=False)
    wk = nc.declare_dram_parameter("wk", [D, DPC], BF16, isOutput=False)
    wvb = nc.declare_dram_parameter("wvb", [D, DPC], BF16, isOutput=False)
    wo = nc.declare_dram_parameter("wo", [DPC, D], BF16, isOutput=False)
    bq = nc.declare_dram_parameter("bq", [DPC, 1], F32, isOutput=False)
    bk = nc.declare_dram_parameter("bk", [DPC, 1], F32, isOutput=False)
    emf = nc.declare_dram_parameter("emf", [B, P, S], BF16, isOutput=False)
    y = nc.declare_dram_parameter("y", [B, S, D], F16, isOutput=True)

    with tile.TileContext(nc) as tc, ExitStack() as ctx:
        # ---- pools ----
        xtb_pool = ctx.enter_context(tc.tile_pool(name="xtb", bufs=16))
        wpool = ctx.enter_context(tc.tile_pool(name="w", bufs=1))
        emf_pool = ctx.enter_context(tc.tile_pool(name="emf", bufs=2))
        qk_pool = ctx.enter_context(tc.tile_pool(name="qk", bufs=2))
        v_pool = ctx.enter_context(tc.tile_pool(name="v", bufs=2))
        et_pool = ctx.enter_context(tc.tile_pool(name="et", bufs=4))
        small = ctx.enter_context(tc.tile_pool(name="small", bufs=3))
        y_pool = ctx.enter_context(tc.tile_pool(name="y", bufs=3))
        ps_s = ctx.enter_context(tc.tile_pool(name="pss", bufs=2, space="PSUM"))
        ps_o = ctx.enter_context(tc.tile_pool(name="pso", bufs=1, space="PSUM"))
        ps_proj = ctx.enter_context(tc.tile_pool(name="psp", bufs=2, space="PSUM"))

        # ---- weights / biases / constants (once) ----
        wq_sb = wpool.tile([P, D], BF16, tag="wq_sb")
        wk_sb = wpool.tile([P, D], BF16, tag="wk_sb")
        wv_sb = wpool.tile([P, D], BF16, tag="wv_sb")
        wo_sb = wpool.tile([P, D], BF16, tag="wo_sb")
        bq_sb = wpool.tile([P, 1], F32, tag="bq_sb")
        bk_sb = wpool.tile([P, 1], F32, tag="bk_sb")
        ones_sb = wpool.tile([P, HD], BF16, tag="ones_sb")
        nc.gpsimd.dma_start(
            out=wq_sb[:].rearrange("p (t m) -> p t m", t=D // P),
            in_=wq.rearrange("(t p) m -> p t m", p=P))
        nc.gpsimd.dma_start(
            out=wk_sb[:].rearrange("p (t m) -> p t m", t=D // P),
            in_=wk.rearrange("(t p) m -> p t m", p=P))
        nc.gpsimd.dma_start(
            out=wv_sb[:].rearrange("p (t m) -> p t m", t=D // P),
            in_=wvb.rearrange("(t p) m -> p t m", p=P))
        nc.gpsimd.dma_start(out=wo_sb[:], in_=wo[:])
        nc.gpsimd.dma_start(out=bq_sb[:], in_=bq[:])
        nc.gpsimd.dma_start(out=bk_sb[:], in_=bk[:])
        nc.vector.memset(ones_sb[:], 1.0)
        wqs = [wq_sb[:, kt * DPC:(kt + 1) * DPC] for kt in range(D // P)]
        wks = [wk_sb[:, kt * DPC:(kt + 1) * DPC] for kt in range(D // P)]
        wvbs = [wv_sb[:, kt * DPC:(kt + 1) * DPC] for kt in range(D // P)]

        for rep in range(repeat):

            def gen_load(b):
                """DMA x^T + emf for batch b; returns tiles."""
                xtbs = []
                for kt in range(D // P):
                    tb = xtb_pool.tile([P, S], BF16, tag="xtb",
                                       name=f"xtb_{rep}_{b}_{kt}")
                    nc.sync.dma_start(out=tb[:],
                                      in_=xtb[b, kt * P:(kt + 1) * P, :])
                    xtbs.append(tb)
                emf_sb = emf_pool.tile([P, S], BF16, tag="emf_sb",
                                       name=f"emf_{rep}_{b}")
                nc.sync.dma_start(out=emf_sb[:], in_=emf[b])
                return xtbs, emf_sb

            def gen_proj(b, xtbs, emf_sb, bt):
                """Q/K/V projections for batch b as labeled pieces.

                Yields after each piece: ('K', ns), ('V', st), ('Q', ns).
                Emission order: K0, Q0, V0, V1, K1, V2..5, K2, V6..9, K3,
                V10..15, Q1..Q3 — so attention can start early.
                """
                def k_piece(ns, half=None):
                    cs = slice(ns * 512, (ns + 1) * 512)
                    if half in (None, 0):
                        psk = ps_proj.tile([P, 512], F32, tag="pp",
                                           name=f"psk_{rep}_{b}_{ns}")
                        bt[f"_psk{ns}"] = psk
                    else:
                        psk = bt[f"_psk{ns}"]
                    kts = range(D // P) if half is None else \
                        range(half * 4, half * 4 + 4)
                    for kt in kts:
                        nc.tensor.matmul(
                            psk[:], lhsT=wks[kt], rhs=xtbs[kt][:, cs],
                            start=(kt == 0), stop=(kt == D // P - 1))
                    if half in (None, 1):
                        nc.vector.tensor_scalar_add(bt["kt"][:, cs], psk[:],
                                                    bk_sb[:])
                        nc.vector.scalar_tensor_tensor(
                            out=bt["k2t"][:, cs], in0=psk[:], scalar=bk_sb[:],
                            in1=emf_sb[:, cs],
                            op0=mybir.AluOpType.add, op1=mybir.AluOpType.mult)

                def q_piece(ns, half=None):
                    cs = slice(ns * 512, (ns + 1) * 512)
                    if half in (None, 0):
                        psq = ps_proj.tile([P, 512], F32, tag="pp",
                                           name=f"psq_{rep}_{b}_{ns}")
                        bt[f"_psq{ns}"] = psq
                    else:
                        psq = bt[f"_psq{ns}"]
                    kts = range(D // P) if half is None else \
                        range(half * 4, half * 4 + 4)
                    for kt in kts:
                        nc.tensor.matmul(
                            psq[:], lhsT=wqs[kt], rhs=xtbs[kt][:, cs],
                            start=(kt == 0), stop=(kt == D // P - 1))
                    if half in (None, 1):
                        nc.vector.tensor_scalar_add(bt["qt"][:, cs], psq[:],
                                                    bq_sb[:])

                def v_piece(st):
                    psv = ps_proj.tile([P, 512], F32, tag="pp",
                                       name=f"psv_{rep}_{b}_{st}")
                    for kt in range(D // P):
                        nc.tensor.matmul(
                            psv[:, 0:DPC],
                            lhsT=xtbs[kt][:, st * P:(st + 1) * P],
                            rhs=wvbs[kt],
                            start=(kt == 0), stop=(kt == D // P - 1))
                    o = st * VSTRIDE
                    nc.vector.tensor_copy(bt["v"][:, o:o + HD], psv[:, 0:HD])
                    nc.vector.tensor_copy(
                        bt["v"][:, o + HD + 1:o + 2 * HD + 1],
                        psv[:, HD:2 * HD])

                nc.vector.memset(bt["v"][:], 1.0)
                k_piece(0); yield ("K", 0)
                q_piece(0); yield ("Q", 0)
                v_piece(0); yield ("V", 0)
                v_piece(1); yield ("V", 1)
                nv = 2
                for ns in (1, 2, 3):
                    k_piece(ns, half=0); yield ("k", ns)
                    k_piece(ns, half=1); yield ("K", ns)
                    for _ in range(4):
                        if nv < NKT:
                            v_piece(nv); yield ("V", nv)
                            nv += 1
                while nv < NKT:
                    v_piece(nv); yield ("V", nv)
                    nv += 1
                for ns in (1, 2, 3):
                    q_piece(ns, half=0); yield ("q", ns)
                    q_piece(ns, half=1); yield ("Q", ns)

            def gen_attn(b, bt):
                """Attention units + lagged output projection, as a generator.

                Yields (j, kt) before emitting unit (j, kt)'s scores so the
                driver can pump projection pieces first.
                """
                qt_sb, kt_sb, k2t_sb = bt["qt"], bt["kt"], bt["k2t"]
                ont_sb, v_sb = bt["ont"], bt["v"]
                units = [(j, kt) for j in range(NSL) for kt in range(NKT)]
                pend = {}
                o_ps = {}
                state = {"st_ready": 0, "st_emitted": 0}
                ready_log = []

                def emit_scores(u):
                    j, kt = u
                    q0 = 512 * j
                    ks = slice(kt * P, (kt + 1) * P)
                    sp = ps_s.tile([P, 1024], F32, tag="sp",
                                   name=f"sp_{rep}_{b}_{j}_{kt}")
                    for (c0, c1, use_k2) in _ranges(j, n_c1[b]):
                        src = k2t_sb if use_k2 else kt_sb
                        nc.tensor.matmul(
                            sp[:, c0 - q0:c1 - q0],
                            lhsT=src[0:HD, ks], rhs=qt_sb[0:HD, c0:c1],
                            start=True, stop=True, tile_position=(0, 0))
                        nc.tensor.matmul(
                            sp[:, 512 + c0 - q0:512 + c1 - q0],
                            lhsT=src[HD:P, ks], rhs=qt_sb[HD:P, c0:c1],
                            start=True, stop=True, tile_position=(64, 0))
                    e = et_pool.tile([P, 1024], BF16, tag="e",
                                     name=f"e_{rep}_{b}_{j}_{kt}")
                    nc.scalar.activation(e[:], sp[:],
                                         mybir.ActivationFunctionType.Exp)
                    pend[u] = e

                def emit_av(u):
                    j, kt = u
                    e = pend.pop(u)
                    if j not in o_ps:
                        o0_t = ps_o.tile([HD + 1, 512], F32, tag="o0",
                                         name=f"o0_{rep}_{b}_{j}")
                        o1_t = ps_o.tile([HD + 1, 512], F32, tag="o1",
                                         name=f"o1_{rep}_{b}_{j}")
                        o_ps[j] = (o0_t, o1_t)
                    o0, o1 = o_ps[j]
                    vo = kt * VSTRIDE
                    nc.tensor.matmul(
                        o0[0:HD + 1, :],
                        lhsT=v_sb[:, vo:vo + HD + 1], rhs=e[:, 0:512],
                        start=(kt == 0), stop=(kt == NKT - 1))
                    nc.tensor.matmul(
                        o1[0:HD + 1, :],
                        lhsT=v_sb[:, vo + HD + 1:vo + 2 * (HD + 1)],
                        rhs=e[:, 512:1024],
                        start=(kt == 0), stop=(kt == NKT - 1))
                    if kt == NKT - 1:
                        emit_norm(j)

                def emit_norm(j):
                    # normalize q-slice j: rows 0:64 of o_h /= row 64 (denom).
                    # recip row -> PE K=1 outer product broadcasts it across
                    # 64 partitions (no DRAM bounce). h0's mul lands directly
                    # in ont rows 0:64; h1's is DMA-shifted to rows 64:128.
                    q0 = 512 * j
                    o0, o1 = o_ps.pop(j)
                    for h, op in ((0, o0), (1, o1)):
                        r = small.tile([HD + 1, 512], BF16, tag=f"r{h}",
                                       name=f"r{h}_{rep}_{b}_{j}")
                        with nc.allow_low_precision(
                                reason="bf16 softmax denom recip: ~0.2%, "
                                       "within the 2e-2 budget"):
                            nc.vector.reciprocal(r[HD:HD + 1, :],
                                                 op[HD:HD + 1, :])
                        bc = ps_proj.tile([P, 512], F32, tag="pp",
                                          name=f"bc{h}_{rep}_{b}_{j}")
                        nc.tensor.matmul(
                            bc[0:HD, :], lhsT=ones_sb[HD:HD + 1, 0:HD],
                            rhs=r[HD:HD + 1, :], start=True, stop=True,
                            tile_position=(64, 0))
                        bcs = small.tile([HD, 512], BF16, tag=f"bcs{h}",
                                         name=f"bcs{h}_{rep}_{b}_{j}")
                        nc.vector.tensor_copy(bcs[:], bc[0:HD, :])
                        if h == 0:
                            nc.vector.tensor_mul(
                                ont_sb[0:HD, q0:q0 + 512],
                                op[0:HD, :], bcs[:])
                        else:
                            tmp = small.tile([HD, 512], BF16, tag="tmp1",
                                             name=f"tmp1_{rep}_{b}_{j}")
                            nc.vector.tensor_mul(
                                tmp[:], op[0:HD, :], bcs[:])
                            nc.gpsimd.dma_start(
                                out=ont_sb[HD:P, q0:q0 + 512],
                                in_=tmp[:])
                    state["st_ready"] = (q0 + 512) // P

                def emit_oproj(upto, tail=False):
                    while state["st_emitted"] < upto:
                        st = state["st_emitted"]
                        ysb = y_pool.tile([P, D], F16, tag="ysb",
                                          name=f"ysb_{rep}_{b}_{st}")
                        for half in range(2):
                            yp = ps_proj.tile([P, 512], F32, tag="pp",
                                              name=f"yp_{rep}_{b}_{st}_{half}")
                            nc.tensor.matmul(
                                yp[:],
                                lhsT=ont_sb[:, st * P:(st + 1) * P],
                                rhs=wo_sb[:, half * 512:(half + 1) * 512],
                                start=True, stop=True)
                            nc.vector.tensor_copy(
                                ysb[:, half * 512:(half + 1) * 512], yp[:])
                        nc.gpsimd.dma_start(out=y[b, st * P:(st + 1) * P, :],
                                          in_=ysb[:])
                        state["st_emitted"] += 1

                for i in range(len(units) + 1):
                    if i < len(units):
                        yield units[i]
                        emit_scores(units[i])
                    if i > 0:
                        emit_av(units[i - 1])
                        ready_log.append((i, state["st_ready"]))
                        lagged = max((s for ii, s in ready_log
                                      if ii <= i - LAG), default=0)
                        emit_oproj(lagged)
                emit_oproj(state["st_ready"], tail=(b == 1))

            def batch_tiles(b):
                return {
                    "qt": qk_pool.tile([P, S], BF16, tag="qt_sb",
                                       name=f"qt_{rep}_{b}"),
                    "kt": qk_pool.tile([P, S], BF16, tag="kt_sb",
                                       name=f"kt_{rep}_{b}"),
                    "k2t": qk_pool.tile([P, S], BF16, tag="k2t_sb",
                                        name=f"k2t_{rep}_{b}"),
                    "ont": qk_pool.tile([P, S], BF16, tag="ont_sb",
                                        name=f"ont_{rep}_{b}"),
                    "v": v_pool.tile([P, NKT * VSTRIDE], BF16, tag="v_sb",
                                     name=f"v_{rep}_{b}"),
                }

            # ---- batch 0: load, then proj pieces pumped just-in-time so
            # attention starts as soon as K0/Q0/V0 are projected ----
            xtbs0, emf0 = gen_load(0)
            bt0 = batch_tiles(0)
            if rep == 0:
                # PE p-state warm-up: ~8.5us of dummy matmuls inside the
                # x-load window so the first projection runs at full clock.
                warm = ps_proj.tile([P, 512], F32, tag="pp", name="warm")
                for wi in range(WARM):
                    nc.tensor.matmul(warm[:], lhsT=wq_sb[:, 0:P],
                                     rhs=wq_sb[:, 0:512],
                                     start=(wi == 0), stop=(wi == WARM - 1))
                warm_rd = small.tile([1, 1], F32, tag="warm_rd")
                nc.vector.tensor_copy(warm_rd[:], warm[0:1, 0:1])
            proj0 = gen_proj(0, xtbs0, emf0, bt0)
            done0 = {"K": set(), "V": set(), "Q": set()}

            def pump(gen, done):
                p = next(gen, None)
                if p is None:
                    return False
                if p[0] in done:
                    done[p[0]].add(p[1])
                return True

            def prereq_met(done, j, kt):
                return (kt // 4 in done["K"] and j in done["Q"]
                        and min(kt + 1, NKT - 1) in done["V"])

            # batch 1 resources, loaded/projected during batch 0's attention
            bt1 = batch_tiles(1)
            state1 = {"loaded": False, "proj": None, "done": False,
                      "d1": {"K": set(), "V": set(), "Q": set()}}

            attn0 = gen_attn(0, bt0)
            for ui, u in enumerate(attn0):
                j, kt = u
                while not prereq_met(done0, j, kt):
                    if not pump(proj0, done0):
                        break
                if ui >= LOAD1_AT and not state1["loaded"]:
                    xtbs1, emf1 = gen_load(1)
                    state1["loaded"] = True
                    state1["proj"] = gen_proj(1, xtbs1, emf1, bt1)
                b0_mostly_done = (len(done0["K"]) == 4
                                  and len(done0["V"]) == NKT
                                  and 1 in done0["Q"])
                if ui >= PUMP_START and ui % PUMP_MOD == 0 \
                        and state1["loaded"] \
                        and not state1["done"] and b0_mostly_done:
                    state1["done"] = not pump(state1["proj"], state1["d1"])
            while pump(proj0, done0):
                pass
            if not state1["loaded"]:
                xtbs1, emf1 = gen_load(1)
                state1["proj"] = gen_proj(1, xtbs1, emf1, bt1)
            while pump(state1["proj"], state1["d1"]):
                pass

            # ---- batch 1 attention ----
            attn1 = gen_attn(1, bt1)
            for _ in attn1:
                pass

    return nc


def _host_prep(x, cause_mask, effect_mask, intervention_strength,
               Wq, bq, Wk, bk, Wv, bv, Wo, bo):
    x = np.asarray(x, dtype=np.float32)
    cause_mask = np.asarray(cause_mask).astype(bool)
    effect_mask = np.asarray(effect_mask).astype(bool)
    s_int = float(np.asarray(intervention_strength))
    Wq = np.asarray(Wq, np.float32); bq = np.asarray(bq, np.float32)
    Wk = np.asarray(Wk, np.float32); bk = np.asarray(bk, np.float32)
    Wv = np.asarray(Wv, np.float32); bv = np.asarray(bv, np.float32)
    Wo = np.asarray(Wo, np.float32); bo = np.asarray(bo, np.float32)

    # host prep: sort tokens by cause_mask (descending) per batch
    perms, n_c1 = [], []
    for b in range(B):
        p = np.argsort(~cause_mask[b], kind="stable")
        perms.append(p)
        n_c1.append(int(cause_mask[b].sum()))
    xp = np.stack([x[b][perms[b]] for b in range(B)])          # [B, S, D]
    xt = np.ascontiguousarray(xp.transpose(0, 2, 1))           # [B, D, S]
    xtb = xt.astype(ml_dtypes.bfloat16)
    emfac = np.stack([
        1.0 - 0.5 * s_int * effect_mask[b][perms[b]].astype(np.float32)
        for b in range(B)])                                    # [B, S]
    emf = np.ascontiguousarray(
        np.broadcast_to(emfac[:, None, :], (B, P, S))).astype(ml_dtypes.bfloat16)

    scale = 1.0 / np.sqrt(np.float32(HD))

    in_maps = []
    for c in range(NCORES):
        cols = slice(c * DPC, (c + 1) * DPC)
        in_maps.append({
            "xtb": xtb, "emf": emf,
            "wq": np.ascontiguousarray(Wq[:, cols] * scale).astype(ml_dtypes.bfloat16),
            "wk": np.ascontiguousarray(Wk[:, cols]).astype(ml_dtypes.bfloat16),
            "wvb": np.ascontiguousarray(Wv[:, cols]).astype(ml_dtypes.bfloat16),
            "wo": np.ascontiguousarray(Wo[cols, :]).astype(ml_dtypes.bfloat16),
            "bq": np.ascontiguousarray((bq[cols] * scale).reshape(DPC, 1)),
            "bk": np.ascontiguousarray(bk[cols].reshape(DPC, 1)),
        })

    bo_eff = bo + bv @ Wo
    return in_maps, perms, tuple(n_c1), bo_eff


def kernel(x, cause_mask, effect_mask, intervention_strength,
           Wq, bq, Wk, bk, Wv, bv, Wo, bo):
    global LAST_RESULTS
    in_maps, perms, n_c1, bo_eff = _host_prep(
        x, cause_mask, effect_mask, intervention_strength,
        Wq, bq, Wk, bk, Wv, bv, Wo, bo)
    nc = _build_program(n_c1)
    nc.finalize()
    LAST_RESULTS = run_bass_kernel_spmd(
        nc, in_maps, core_ids=list(range(NCORES)), trace=TRACE)

    y = np.zeros((B, S, D), np.float32)
    for c in range(NCORES):
        y += LAST_RESULTS.results[c]["y"].astype(np.float32)
    y += bo_eff[None, None, :]
    out = np.empty_like(y)
    for b in range(B):
        out[b][perms[b]] = y[b]  # undo the token sort
    return out


# revision 37
# speedup vs baseline: 1.0226x; 1.0048x over previous
"""Causal-intervention attention on 8 trn2 cores.

Sharding: head-parallel. Core c computes heads {2c, 2c+1} for BOTH batches.
Each core emits a partial output y_c = ctx_c @ Wo[rows_c] in fp16; the host
sums the 8 partials and adds the (folded) bias.

Mask handling: tokens are sorted by cause_mask on the host (per batch).
scores * (1 - 0.5*s*cm[q]*em[k]) is exact by using an em-scaled copy of K^T
(K2) for cm=1 queries and plain K^T for cm=0 queries. Query slices are a
uniform 512 wide; the cm boundary splits only the scores matmul col-range.

Per-unit structure (unit = (q-slice j, key-tile kt)): both heads' scores go
into ONE [128, 1024] PSUM tile (h0 cols 0:512, h1 cols 512:1024) via
tile_position (0,0)/(64,0), so a single full-width exp serves both heads.
Score PSUM is double-buffered so PE never waits on ACT.

Softmax: exp without max subtraction (scores ~ N(0,1), safe in fp32).
Denominator from a ones column appended to V (M=65 AV matmul). The
reciprocal row is broadcast across partitions with a K=1 PE outer-product
(no DRAM bounce). Normalization is applied to the attention output before
the output projection; h1's rows are DMA-shifted to partitions 64:128.

All matmul operands bf16; accumulation fp32 in PSUM.
"""

import numpy as np
import ml_dtypes
from contextlib import ExitStack

import concourse.bass as bass
from concourse import bacc
import concourse.mybir as mybir
import concourse.tile as tile
from concourse.bass_utils import run_bass_kernel_spmd

B, S, D, H = 2, 2048, 1024, 16
HD = D // H  # 64
NCORES = 8
HPC = H // NCORES  # heads per core = 2
DPC = HPC * HD  # head-dim cols per core = 128
P = 128

F32 = mybir.dt.float32
F16 = mybir.dt.float16
BF16 = mybir.dt.bfloat16

NKT = S // P            # 16 key tiles of 128
NSL = S // 512          # 4 query slices of 512
VSTRIDE = 2 * (HD + 1)  # V' storage per k-tile: [V_h0|1|V_h1|1] = 130

LAST_RESULTS = None  # BassKernelResults of the most recent run (for test.py)
TRACE = False

# schedule pacing knobs (sim-swept)
PUMP_START = 12   # unit index to start pumping batch-1 proj pieces
PUMP_MOD = 2      # pump one piece every PUMP_MOD units
WARM = 24         # PE p-state warm-up matmuls
LOAD1_AT = 4      # unit index to issue batch-1 x/emf loads
LAG = 2           # oproj lag in units


def _ranges(j, boundary):
    """Column ranges of q-slice j=[512j, 512j+512) split at the cm=1/cm=0
    boundary. Returns [(c0, c1, use_k2)]."""
    q0, q1 = 512 * j, 512 * (j + 1)
    b = min(max(boundary, q0), q1)
    out = []
    if b > q0:
        out.append((q0, b, True))
    if q1 > b:
        out.append((b, q1, False))
    return out


def _build_program(n_c1, repeat=1):
    """n_c1: tuple of per-batch cm=1 counts (compile-time structure).
    repeat>1 re-runs the whole computation (timing experiments only)."""
    nc = bacc.Bacc(None, target_bir_lowering=False)

    xtb = nc.declare_dram_parameter("xtb", [B, D, S], BF16, isOutput=False)
    wq = nc.declare_dram_parameter("wq", [D, DPC], BF16, isOutput=False)
    wk = nc.declare_dram_parameter("wk", [D, DPC], BF16, isOutput=False)
    wvb = nc.declare_dram_parameter("wvb", [D, DPC], BF16, isOutput=False)
    wo = nc.declare_dram_parameter("wo", [DPC, D], BF16, isOutput=False)
    bq = nc.declare_dram_parameter("bq", [DPC, 1], F32, isOutput=False)
    bk = nc.declare_dram_parameter("bk", [DPC, 1], F32, isOutput=False)
    emf = nc.declare_dram_parameter("emf", [B, P, S], BF16, isOutput=False)
    y = nc.declare_dram_parameter("y", [B, S, D], F16, isOutput=True)

    with tile.TileContext(nc) as tc, ExitStack() as ctx:
        # ---- pools ----
        xtb_pool = ctx.enter_context(tc.tile_pool(name="xtb", bufs=16))
        wpool = ctx.enter_context(tc.tile_pool(name="w", bufs=1))
        emf_pool = ctx.enter_context(tc.tile_pool(name="emf", bufs=2))
        qk_pool = ctx.enter_context(tc.tile_pool(name="qk", bufs=2))
        v_pool = ctx.enter_context(tc.tile_pool(name="v", bufs=2))
        et_pool = ctx.enter_context(tc.tile_pool(name="et", bufs=4))
        small = ctx.enter_context(tc.tile_pool(name="small", bufs=3))
        y_pool = ctx.enter_context(tc.tile_pool(name="y", bufs=4))
        ps_s = ctx.enter_context(tc.tile_pool(name="pss", bufs=2, space="PSUM"))
        ps_o = ctx.enter_context(tc.tile_pool(name="pso", bufs=1, space="PSUM"))
        ps_proj = ctx.enter_context(tc.tile_pool(name="psp", bufs=2, space="PSUM"))

        # ---- weights / biases / constants (once) ----
        wq_sb = wpool.tile([P, D], BF16, tag="wq_sb")
        wk_sb = wpool.tile([P, D], BF16, tag="wk_sb")
        wv_sb = wpool.tile([P, D], BF16, tag="wv_sb")
        wo_sb = wpool.tile([P, D], BF16, tag="wo_sb")
        bq_sb = wpool.tile([P, 1], F32, tag="bq_sb")
        bk_sb = wpool.tile([P, 1], F32, tag="bk_sb")
        ones_sb = wpool.tile([P, HD], BF16, tag="ones_sb")
        nc.gpsimd.dma_start(
            out=wq_sb[:].rearrange("p (t m) -> p t m", t=D // P),
            in_=wq.rearrange("(t p) m -> p t m", p=P))
        nc.gpsimd.dma_start(
            out=wk_sb[:].rearrange("p (t m) -> p t m", t=D // P),
            in_=wk.rearrange("(t p) m -> p t m", p=P))
        nc.gpsimd.dma_start(
            out=wv_sb[:].rearrange("p (t m) -> p t m", t=D // P),
            in_=wvb.rearrange("(t p) m -> p t m", p=P))
        nc.gpsimd.dma_start(out=wo_sb[:], in_=wo[:])
        nc.gpsimd.dma_start(out=bq_sb[:], in_=bq[:])
        nc.gpsimd.dma_start(out=bk_sb[:], in_=bk[:])
        nc.vector.memset(ones_sb[:], 1.0)
        wqs = [wq_sb[:, kt * DPC:(kt + 1) * DPC] for kt in range(D // P)]
        wks = [wk_sb[:, kt * DPC:(kt + 1) * DPC] for kt in range(D // P)]
        wvbs = [wv_sb[:, kt * DPC:(kt + 1) * DPC] for kt in range(D // P)]

        for rep in range(repeat):

            def gen_load(b):
                """DMA x^T + emf for batch b; returns tiles."""
                xtbs = []
                for kt in range(D // P):
                    tb = xtb_pool.tile([P, S], BF16, tag="xtb",
                                       name=f"xtb_{rep}_{b}_{kt}")
                    nc.sync.dma_start(out=tb[:],
                                      in_=xtb[b, kt * P:(kt + 1) * P, :])
                    xtbs.append(tb)
                emf_sb = emf_pool.tile([P, S], BF16, tag="emf_sb",
                                       name=f"emf_{rep}_{b}")
                nc.sync.dma_start(out=emf_sb[:], in_=emf[b])
                return xtbs, emf_sb

            def gen_proj(b, xtbs, emf_sb, bt):
                """Q/K/V projections for batch b as labeled pieces.

                Yields after each piece: ('K', ns), ('V', st), ('Q', ns).
                Emission order: K0, Q0, V0, V1, K1, V2..5, K2, V6..9, K3,
                V10..15, Q1..Q3 — so attention can start early.
                """
                def k_piece(ns, half=None):
                    cs = slice(ns * 512, (ns + 1) * 512)
                    if half in (None, 0):
                        psk = ps_proj.tile([P, 512], F32, tag="pp",
                                           name=f"psk_{rep}_{b}_{ns}")
                        bt[f"_psk{ns}"] = psk
                    else:
                        psk = bt[f"_psk{ns}"]
                    kts = range(D // P) if half is None else \
                        range(half * 4, half * 4 + 4)
                    for kt in kts:
                        nc.tensor.matmul(
                            psk[:], lhsT=wks[kt], rhs=xtbs[kt][:, cs],
                            start=(kt == 0), stop=(kt == D // P - 1))
                    if half in (None, 1):
                        nc.vector.tensor_scalar_add(bt["kt"][:, cs], psk[:],
                                                    bk_sb[:])
                        nc.vector.scalar_tensor_tensor(
                            out=bt["k2t"][:, cs], in0=psk[:], scalar=bk_sb[:],
                            in1=emf_sb[:, cs],
                            op0=mybir.AluOpType.add, op1=mybir.AluOpType.mult)

                def q_piece(ns, half=None):
                    cs = slice(ns * 512, (ns + 1) * 512)
                    if half in (None, 0):
                        psq = ps_proj.tile([P, 512], F32, tag="pp",
                                           name=f"psq_{rep}_{b}_{ns}")
                        bt[f"_psq{ns}"] = psq
                    else:
                        psq = bt[f"_psq{ns}"]
                    kts = range(D // P) if half is None else \
                        range(half * 4, half * 4 + 4)
                    for kt in kts:
                        nc.tensor.matmul(
                            psq[:], lhsT=wqs[kt], rhs=xtbs[kt][:, cs],
                            start=(kt == 0), stop=(kt == D // P - 1))
                    if half in (None, 1):
                        nc.vector.tensor_scalar_add(bt["qt"][:, cs], psq[:],
                                                    bq_sb[:])

                def v_piece(st):
                    psv = ps_proj.tile([P, 512], F32, tag="pp",
                                       name=f"psv_{rep}_{b}_{st}")
                    for kt in range(D // P):
                        nc.tensor.matmul(
                            psv[:, 0:DPC],
                            lhsT=xtbs[kt][:, st * P:(st + 1) * P],
                            rhs=wvbs[kt],
                            start=(kt == 0), stop=(kt == D // P - 1))
                    o = st * VSTRIDE
                    nc.vector.tensor_copy(bt["v"][:, o:o + HD], psv[:, 0:HD])
                    nc.vector.tensor_copy(
                        bt["v"][:, o + HD + 1:o + 2 * HD + 1],
                        psv[:, HD:2 * HD])

                nc.vector.memset(bt["v"][:], 1.0)
                k_piece(0); yield ("K", 0)
                q_piece(0); yield ("Q", 0)
                v_piece(0); yield ("V", 0)
                v_piece(1); yield ("V", 1)
                nv = 2
                for ns in (1, 2, 3):
                    k_piece(ns, half=0); yield ("k", ns)
                    k_piece(ns, half=1); yield ("K", ns)
                    for _ in range(4):
                        if nv < NKT:
                            v_piece(nv); yield ("V", nv)
                            nv += 1
                while nv < NKT:
                    v_piece(nv); yield ("V", nv)
                    nv += 1
                for ns in (1, 2, 3):
                    q_piece(ns, half=0); yield ("q", ns)
                    q_piece(ns, half=1); yield ("Q", ns)

            def gen_attn(b, bt):
                """Attention units + lagged output projection, as a generator.

                Yields (j, kt) before emitting unit (j, kt)'s scores so the
                driver can pump projection pieces first.
                """
                qt_sb, kt_sb, k2t_sb = bt["qt"], bt["kt"], bt["k2t"]
                ont_sb, v_sb = bt["ont"], bt["v"]
                units = [(j, kt) for j in range(NSL) for kt in range(NKT)]
                pend = {}
                o_ps = {}
                state = {"st_ready": 0, "st_emitted": 0}
                ready_log = []

                def emit_scores(u):
                    j, kt = u
                    q0 = 512 * j
                    ks = slice(kt * P, (kt + 1) * P)
                    sp = ps_s.tile([P, 1024], F32, tag="sp",
                                   name=f"sp_{rep}_{b}_{j}_{kt}")
                    for (c0, c1, use_k2) in _ranges(j, n_c1[b]):
                        src = k2t_sb if use_k2 else kt_sb
                        nc.tensor.matmul(
                            sp[:, c0 - q0:c1 - q0],
                            lhsT=src[0:HD, ks], rhs=qt_sb[0:HD, c0:c1],
                            start=True, stop=True, tile_position=(0, 0))
                        nc.tensor.matmul(
                            sp[:, 512 + c0 - q0:512 + c1 - q0],
                            lhsT=src[HD:P, ks], rhs=qt_sb[HD:P, c0:c1],
                            start=True, stop=True, tile_position=(64, 0))
                    e = et_pool.tile([P, 1024], BF16, tag="e",
                                     name=f"e_{rep}_{b}_{j}_{kt}")
                    nc.scalar.activation(e[:], sp[:],
                                         mybir.ActivationFunctionType.Exp)
                    pend[u] = e

                def emit_av(u):
                    j, kt = u
                    e = pend.pop(u)
                    if j not in o_ps:
                        o0_t = ps_o.tile([HD + 1, 512], F32, tag="o0",
                                         name=f"o0_{rep}_{b}_{j}")
                        o1_t = ps_o.tile([HD + 1, 512], F32, tag="o1",
                                         name=f"o1_{rep}_{b}_{j}")
                        o_ps[j] = (o0_t, o1_t)
                    o0, o1 = o_ps[j]
                    vo = kt * VSTRIDE
                    nc.tensor.matmul(
                        o0[0:HD + 1, :],
                        lhsT=v_sb[:, vo:vo + HD + 1], rhs=e[:, 0:512],
                        start=(kt == 0), stop=(kt == NKT - 1))
                    nc.tensor.matmul(
                        o1[0:HD + 1, :],
                        lhsT=v_sb[:, vo + HD + 1:vo + 2 * (HD + 1)],
                        rhs=e[:, 512:1024],
                        start=(kt == 0), stop=(kt == NKT - 1))
                    if kt == NKT - 1:
                        emit_norm(j)

                def emit_norm(j, cols=(0, 512), pop=None):
                    # normalize q-slice j: rows 0:64 of o_h /= row 64 (denom).
                    # recip row -> PE K=1 outer product broadcasts it across
                    # 64 partitions (no DRAM bounce). h0's mul lands directly
                    # in ont rows 0:64; h1's is DMA-shifted to rows 64:128.
                    q0 = 512 * j
                    c0, c1 = cols
                    w = c1 - c0
                    if pop is None:
                        pop = (c1 == 512)
                    if pop:
                        o0, o1 = o_ps.pop(j)
                    else:
                        o0, o1 = o_ps[j]
                    for h, op in ((0, o0), (1, o1)):
                        r = small.tile([HD + 1, 512], BF16, tag=f"r{h}",
                                       name=f"r{h}_{rep}_{b}_{j}_{c0}")
                        with nc.allow_low_precision(
                                reason="bf16 softmax denom recip: ~0.2%, "
                                       "within the 2e-2 budget"):
                            nc.vector.reciprocal(r[HD:HD + 1, 0:w],
                                                 op[HD:HD + 1, c0:c1])
                        bc = ps_proj.tile([P, 512], F32, tag="pp",
                                          name=f"bc{h}_{rep}_{b}_{j}_{c0}")
                        nc.tensor.matmul(
                            bc[0:HD, 0:w], lhsT=ones_sb[HD:HD + 1, 0:HD],
                            rhs=r[HD:HD + 1, 0:w], start=True, stop=True,
                            tile_position=(64, 0))
                        bcs = small.tile([HD, 512], BF16, tag=f"bcs{h}",
                                         name=f"bcs{h}_{rep}_{b}_{j}_{c0}")
                        nc.vector.tensor_copy(bcs[0:HD, 0:w], bc[0:HD, 0:w])
                        if h == 0:
                            nc.vector.tensor_mul(
                                ont_sb[0:HD, q0 + c0:q0 + c1],
                                op[0:HD, c0:c1], bcs[0:HD, 0:w])
                        else:
                            tmp = small.tile([HD, 512], BF16, tag="tmp1",
                                             name=f"tmp1_{rep}_{b}_{j}_{c0}")
                            nc.vector.tensor_mul(
                                tmp[0:HD, 0:w], op[0:HD, c0:c1],
                                bcs[0:HD, 0:w])
                            nc.gpsimd.dma_start(
                                out=ont_sb[HD:P, q0 + c0:q0 + c1],
                                in_=tmp[0:HD, 0:w])
                    if c1 == 512:
                        state["st_ready"] = (q0 + 512) // P

                def emit_oproj(upto, tail=False, limit=99):
                    while state["st_emitted"] < upto and limit > 0:
                        limit -= 1
                        st = state["st_emitted"]
                        ysb = y_pool.tile([P, D], F16, tag="ysb",
                                          name=f"ysb_{rep}_{b}_{st}")
                        for half in range(2):
                            yp = ps_proj.tile([P, 512], F32, tag="pp",
                                              name=f"yp_{rep}_{b}_{st}_{half}")
                            nc.tensor.matmul(
                                yp[:],
                                lhsT=ont_sb[:, st * P:(st + 1) * P],
                                rhs=wo_sb[:, half * 512:(half + 1) * 512],
                                start=True, stop=True)
                            if tail and half == 1:
                                nc.scalar.activation(
                                    ysb[:, half * 512:(half + 1) * 512],
                                    yp[:],
                                    mybir.ActivationFunctionType.Copy)
                            else:
                                nc.vector.tensor_copy(
                                    ysb[:, half * 512:(half + 1) * 512],
                                    yp[:])
                        nc.gpsimd.dma_start(out=y[b, st * P:(st + 1) * P, :],
                                          in_=ysb[:])
                        state["st_emitted"] += 1

                for i in range(len(units) + 1):
                    if i < len(units):
                        yield units[i]
                        emit_scores(units[i])
                    if i > 0:
                        emit_av(units[i - 1])
                        ready_log.append((i, state["st_ready"]))
                        lagged = max((s for ii, s in ready_log
                                      if ii <= i - LAG), default=0)
                        emit_oproj(lagged)
                emit_oproj(state["st_ready"], tail=(b == 1))

            def batch_tiles(b):
                return {
                    "qt": qk_pool.tile([P, S], BF16, tag="qt_sb",
                                       name=f"qt_{rep}_{b}"),
                    "kt": qk_pool.tile([P, S], BF16, tag="kt_sb",
                                       name=f"kt_{rep}_{b}"),
                    "k2t": qk_pool.tile([P, S], BF16, tag="k2t_sb",
                                        name=f"k2t_{rep}_{b}"),
                    "ont": qk_pool.tile([P, S], BF16, tag="ont_sb",
                                        name=f"ont_{rep}_{b}"),
                    "v": v_pool.tile([P, NKT * VSTRIDE], BF16, tag="v_sb",
                                     name=f"v_{rep}_{b}"),
                }

            # ---- batch 0: load, then proj pieces pumped just-in-time so
            # attention starts as soon as K0/Q0/V0 are projected ----
            xtbs0, emf0 = gen_load(0)
            bt0 = batch_tiles(0)
            if rep == 0:
                # PE p-state warm-up: ~8.5us of dummy matmuls inside the
                # x-load window so the first projection runs at full clock.
                warm = ps_proj.tile([P, 512], F32, tag="pp", name="warm")
                for wi in range(WARM):
                    nc.tensor.matmul(warm[:], lhsT=wq_sb[:, 0:P],
                                     rhs=wq_sb[:, 0:512],
                                     start=(wi == 0), stop=(wi == WARM - 1))
                warm_rd = small.tile([1, 1], F32, tag="warm_rd")
                nc.vector.tensor_copy(warm_rd[:], warm[0:1, 0:1])
            proj0 = gen_proj(0, xtbs0, emf0, bt0)
            done0 = {"K": set(), "V": set(), "Q": set()}

            def pump(gen, done):
                p = next(gen, None)
                if p is None:
                    return False
                if p[0] in done:
                    done[p[0]].add(p[1])
                return True

            def prereq_met(done, j, kt):
                return (kt // 4 in done["K"] and j in done["Q"]
                        and min(kt + 1, NKT - 1) in done["V"])

            # batch 1 resources, loaded/projected during batch 0's attention
            bt1 = batch_tiles(1)
            state1 = {"loaded": False, "proj": None, "done": False,
                      "d1": {"K": set(), "V": set(), "Q": set()}}

            attn0 = gen_attn(0, bt0)
            for ui, u in enumerate(attn0):
                j, kt = u
                while not prereq_met(done0, j, kt):
                    if not pump(proj0, done0):
                        break
                if ui >= LOAD1_AT and not state1["loaded"]:
                    xtbs1, emf1 = gen_load(1)
                    state1["loaded"] = True
                    state1["proj"] = gen_proj(1, xtbs1, emf1, bt1)
                b0_mostly_done = (len(done0["K"]) == 4
                                  and len(done0["V"]) == NKT
                                  and 1 in done0["Q"])
                if ui >= PUMP_START and ui % PUMP_MOD == 0 \
                        and state1["loaded"] \
                        and not state1["done"] and b0_mostly_done:
                    state1["done"] = not pump(state1["proj"], state1["d1"])
            while pump(proj0, done0):
                pass
            if not state1["loaded"]:
                xtbs1, emf1 = gen_load(1)
                state1["proj"] = gen_proj(1, xtbs1, emf1, bt1)
            while pump(state1["proj"], state1["d1"]):
                pass

            # ---- batch 1 attention ----
            attn1 = gen_attn(1, bt1)
            for _ in attn1:
                pass

    return nc


def _host_prep(x, cause_mask, effect_mask, intervention_strength,
               Wq, bq, Wk, bk, Wv, bv, Wo, bo):
    x = np.asarray(x, dtype=np.float32)
    cause_mask = np.asarray(cause_mask).astype(bool)
    effect_mask = np.asarray(effect_mask).astype(bool)
    s_int = float(np.asarray(intervention_strength))
    Wq = np.asarray(Wq, np.float32); bq = np.asarray(bq, np.float32)
    Wk = np.asarray(Wk, np.float32); bk = np.asarray(bk, np.float32)
    Wv = np.asarray(Wv, np.float32); bv = np.asarray(bv, np.float32)
    Wo = np.asarray(Wo, np.float32); bo = np.asarray(bo, np.float32)

    # host prep: sort tokens by cause_mask (descending) per batch
    perms, n_c1 = [], []
    for b in range(B):
        p = np.argsort(~cause_mask[b], kind="stable")
        perms.append(p)
        n_c1.append(int(cause_mask[b].sum()))
    xp = np.stack([x[b][perms[b]] for b in range(B)])          # [B, S, D]
    xt = np.ascontiguousarray(xp.transpose(0, 2, 1))           # [B, D, S]
    xtb = xt.astype(ml_dtypes.bfloat16)
    emfac = np.stack([
        1.0 - 0.5 * s_int * effect_mask[b][perms[b]].astype(np.float32)
        for b in range(B)])                                    # [B, S]
    emf = np.ascontiguousarray(
        np.broadcast_to(emfac[:, None, :], (B, P, S))).astype(ml_dtypes.bfloat16)

    scale = 1.0 / np.sqrt(np.float32(HD))

    in_maps = []
    for c in range(NCORES):
        cols = slice(c * DPC, (c + 1) * DPC)
        in_maps.append({
            "xtb": xtb, "emf": emf,
            "wq": np.ascontiguousarray(Wq[:, cols] * scale).astype(ml_dtypes.bfloat16),
            "wk": np.ascontiguousarray(Wk[:, cols]).astype(ml_dtypes.bfloat16),
            "wvb": np.ascontiguousarray(Wv[:, cols]).astype(ml_dtypes.bfloat16),
            "wo": np.ascontiguousarray(Wo[cols, :]).astype(ml_dtypes.bfloat16),
            "bq": np.ascontiguousarray((bq[cols] * scale).reshape(DPC, 1)),
            "bk": np.ascontiguousarray(bk[cols].reshape(DPC, 1)),
        })

    bo_eff = bo + bv @ Wo
    return in_maps, perms, tuple(n_c1), bo_eff


def kernel(x, cause_mask, effect_mask, intervention_strength,
           Wq, bq, Wk, bk, Wv, bv, Wo, bo):
    global LAST_RESULTS
    in_maps, perms, n_c1, bo_eff = _host_prep(
        x, cause_mask, effect_mask, intervention_strength,
        Wq, bq, Wk, bk, Wv, bv, Wo, bo)
    nc = _build_program(n_c1)
    nc.finalize()
    LAST_RESULTS = run_bass_kernel_spmd(
        nc, in_maps, core_ids=list(range(NCORES)), trace=TRACE)

    y = np.zeros((B, S, D), np.float32)
    for c in range(NCORES):
        y += LAST_RESULTS.results[c]["y"].astype(np.float32)
    y += bo_eff[None, None, :]
    out = np.empty_like(y)
    for b in range(B):
        out[b][perms[b]] = y[b]  # undo the token sort
    return out
